# revision 1
# baseline (speedup 1.0000x reference)
"""MoE top-2/8 expert-parallel kernel for TRN2 (8 cores).

Sharding: expert weights sharded (core c == expert c); x replicated to all
cores pre-cast to bf16 on host (xh) plus per-core hi/lo slices (xhs/xls)
for the fp32-accurate split-bf16 router. No x AllGather on device.

Per-core pipeline:
  1. Router on own 512-token slice in split-bf16 (3-term):
     logits = xh@rkh + xl@rkh + xh@rkl.  AllGather logits (tiny).
  2. Top-2 per token via DVE max8/max_index; w1 = sigmoid((E1-E2)/Z);
     per-expert compaction via triangular-matmul cumsum -> slot per token.
     A second cumsum over the one-hot expert grid [P, E, TO] gives, for
     every (token, expert), the token's rank within the (expert, home)
     bucket -- used for the all-to-all return addressing.
  3. Indirect HW-DGE row-gathers from local xh (9 x 128 rows) + PE
     transposes into xeT [128, D/128, C].
  4. gate/up matmuls (bf16, fp32 accum) -> silu*up -> fuse bf16
     down matmul -> scale by per-slot gate weight -> indirect row-scatter
     (HW DGE, plain write) into sendbuf[home*CAPR + rank].
  5. AllToAll(bf16) over 8 cores: bucket (e -> h) lands on home h at rows
     [e*CAPR + rank].  Home gathers its 512 tokens x 2 expert rows via
     eight indirect row-gathers (row ids extracted from a global table
     with eid-computed indirect reads), adds the pairs, writes fp32 out.
     No dense accumulator, no zeroing, no ReduceScatter.
"""

import numpy as np
import concourse.bass as bass
import concourse.mybir as mybir
import concourse.tile as tile
from concourse import bacc
from concourse.masks import make_identity, make_upper_triangular

P = 128
T, D, F, E = 4096, 2048, 1024, 8
TS = T // 8          # tokens per core slice
GC = 384             # gather chunk (3 chunks = C)
C = 3 * GC           # 1152 per-expert token capacity (measured max 1058)
NTRASH = 64          # trash rows / slots
CAPR = 176           # per (expert, home) bucket capacity (measured max 147)
SBR = 8 * CAPR       # all-to-all buffer rows
dt = mybir.dt
AF = mybir.ActivationFunctionType
ALU = mybir.AluOpType

TO = T // P   # 32 token columns (t = o*128 + p)
SO = TS // P  # 4 token columns per slice
KO = D // P   # 16 contraction tiles over D
FO = F // P   # 8 f-tiles
CM = C // P   # 9 slot tiles
C16 = C // 16           # 72


def build(n_cores: int = 8, repeat: int = 1, stage: str = "full"):
    TS = T // n_cores
    SO = TS // P
    nc = bacc.Bacc("TRN2", target_bir_lowering=False, debug=False,
                   num_devices=n_cores)

    xh = nc.dram_tensor("xh", [T, D], dt.bfloat16, kind="ExternalInput")
    xhs = nc.dram_tensor("xhs", [TS, D], dt.bfloat16, kind="ExternalInput")
    xls = nc.dram_tensor("xls", [TS, D], dt.bfloat16, kind="ExternalInput")
    rkh = nc.dram_tensor("rkh", [D, E], dt.bfloat16, kind="ExternalInput")
    rkl = nc.dram_tensor("rkl", [D, E], dt.bfloat16, kind="ExternalInput")
    wg = nc.dram_tensor("wg", [D, F], dt.bfloat16, kind="ExternalInput")
    wu = nc.dram_tensor("wu", [D, F], dt.bfloat16, kind="ExternalInput")
    wd = nc.dram_tensor("wd", [F, D], dt.bfloat16, kind="ExternalInput")
    eid = nc.dram_tensor("eid", [P, 1], dt.float32, kind="ExternalInput")
    out = nc.dram_tensor("out", [TS, D], dt.float32, kind="ExternalOutput")

    with tile.TileContext(nc) as tc:
        with (
            tc.tile_pool(name="dram", bufs=1, space="DRAM") as dram,
            tc.tile_pool(name="consts", bufs=1) as consts,
            tc.tile_pool(name="wpool", bufs=1) as wpool,
            tc.tile_pool(name="main", bufs=1) as main,
        ):
          for _rep in range(repeat):
            # ---------------- DRAM scratch ----------------
            lg_slice = dram.tile([TS, E], dt.float32)
            lg_full = dram.tile([T, E], dt.float32, addr_space="Shared")
            slotd = dram.tile([T], dt.int16)
            ilist2 = dram.tile([C + NTRASH, 64], dt.float32)
            gtab4 = dram.tile([T // 4, 8], dt.float32)
            sendbuf = dram.tile([SBR + P, D], dt.bfloat16)
            recvbuf = dram.tile([SBR, D], dt.bfloat16)

            # ---------------- constants ----------------
            ident_bf = consts.tile([P, P], dt.bfloat16)
            make_identity(nc, ident_bf[:])
            ident_f32 = consts.tile([P, P], dt.float32)
            make_identity(nc, ident_f32[:])
            triu_bf = consts.tile([P, P], dt.bfloat16)
            make_upper_triangular(nc, triu_bf[:], val=1.0, diag=True)
            eid_sb = consts.tile([P, 1], dt.float32)
            nc.sync.dma_start(eid_sb[:], eid[:])
            ones_bf = consts.tile([P, 1], dt.bfloat16)
            nc.vector.memset(ones_bf[:], 1.0)
            pidx = consts.tile([P, 1], dt.int32)
            nc.gpsimd.iota(pidx[:], pattern=[[0, 1]], base=0,
                           channel_multiplier=1)

            # -------- weights: bf16 straight into SBUF (start early) -----
            wg_sb = wpool.tile([P, KO, F], dt.bfloat16)
            wu_sb = wpool.tile([P, KO, F], dt.bfloat16)
            wd_sb = wpool.tile([P, FO, D], dt.bfloat16)
            wg_r = wg[:].rearrange("(ko p) f -> p ko f", p=P)
            wu_r = wu[:].rearrange("(ko p) f -> p ko f", p=P)
            wd_r = wd[:].rearrange("(fo p) d -> p fo d", p=P)
            for ko in range(KO):
                nc.gpsimd.dma_start(wg_sb[:, ko], wg_r[:, ko])
                nc.gpsimd.dma_start(wu_sb[:, ko], wu_r[:, ko])
            for fo in range(FO):
                nc.gpsimd.dma_start(wd_sb[:, fo], wd_r[:, fo])

            # init ilist2 rows to zeros early (count field -> empty detect);
            # only depends on a constant memset, runs under the router
            init_sb = consts.tile([P, 16], dt.float32, tag="init_sb",
                                  name="init_sb")
            nc.vector.memset(init_sb[:], 0.0)
            nil = C + NTRASH
            for r0 in range(0, nil, P):
                rr = min(P, nil - r0)
                nc.scalar.dma_start(ilist2[r0:r0 + rr, :16], init_sb[:rr, :])

            # ---------------- router phase (own 512-token slice) ---------
            with tc.tile_pool(name="route", bufs=1) as route, \
                 tc.tile_pool(name="ps_route", bufs=2, space="PSUM") as psr:
                rkh_sb = route.tile([P, KO, E], dt.bfloat16)
                rkl_sb = route.tile([P, KO, E], dt.bfloat16)
                nc.sync.dma_start(rkh_sb[:],
                                  rkh[:].rearrange("(ko p) e -> p ko e", p=P))
                nc.sync.dma_start(rkl_sb[:],
                                  rkl[:].rearrange("(ko p) e -> p ko e", p=P))

                lg_sb = route.tile([P, SO, E], dt.float32)
                xhs_r = xhs[:].rearrange("(o p) d -> p o d", p=P)
                xls_r = xls[:].rearrange("(o p) d -> p o d", p=P)
                xh_sb = route.tile([P, SO, D], dt.bfloat16)
                xl_sb = route.tile([P, SO, D], dt.bfloat16)
                for o in range(SO):
                    nc.sync.dma_start(xh_sb[:, o], xhs_r[:, o])
                    nc.sync.dma_start(xl_sb[:, o], xls_r[:, o])

                xhT = route.tile([P, KO, SO * P], dt.bfloat16)
                xlT = route.tile([P, KO, SO * P], dt.bfloat16)
                for src, dstT in ((xh_sb, xhT), (xl_sb, xlT)):
                    for ko in range(KO):
                        for o in range(SO):
                            pt = psr.tile([P, P], dt.bfloat16, tag="tp",
                                          name="pt")
                            nc.tensor.transpose(
                                pt[:], src[:, o, ko * P:(ko + 1) * P],
                                ident_bf[:])
                            nc.scalar.copy(
                                dstT[:, ko, o * P:(o + 1) * P], pt[:])

                ps_l = psr.tile([E, SO * P], dt.float32, tag="psl",
                                name="ps_l")
                steps = []
                for ko in range(KO):
                    steps.append((rkh_sb[:, ko], xhT[:, ko]))
                    steps.append((rkl_sb[:, ko], xhT[:, ko]))
                    steps.append((rkh_sb[:, ko], xlT[:, ko]))
                for i, (lhsT, rhs) in enumerate(steps):
                    nc.tensor.matmul(ps_l[:], lhsT, rhs, start=(i == 0),
                                     stop=(i == len(steps) - 1))
                lgT_sb = route.tile([E, SO * P], dt.float32, tag="lgT",
                                    name="lgT_sb")
                nc.vector.tensor_copy(lgT_sb[:], ps_l[:])
                for o in range(SO):
                    pt2 = psr.tile([P, E], dt.float32, tag="tp2", name="pt2")
                    nc.tensor.transpose(pt2[:], lgT_sb[:, o * P:(o + 1) * P],
                                        ident_f32[:E, :E])
                    nc.vector.tensor_copy(lg_sb[:, o], pt2[:])
                nc.sync.dma_start(
                    lg_slice[:].rearrange("(o p) e -> p o e", p=P), lg_sb[:])

            # ---------------- collectives: AllGather logits --------------
            if n_cores > 1:
                nc.gpsimd.collective_compute(
                    "AllGather", ALU.bypass,
                    ins=[lg_slice[:].opt()], outs=[lg_full[:].opt()],
                    replica_groups=[list(range(n_cores))])
            else:
                nc.sync.dma_start(lg_full[:], lg_slice[:])

            # ---------------- routing math (global [P, TO] grid) ---------
            L = main.tile([P, TO, E], dt.float32)
            nc.sync.dma_start(L[:], lg_full[:].rearrange("(o p) e -> p o e", p=P))
            # batched top-2 via segmented reduce_max + arithmetic argmax
            # (tie-break = lowest index, matching jax top_k)
            BIGC = 1.0e4
            iotaE = main.tile([P, 1, E], dt.float32)
            for e in range(E):
                nc.vector.memset(iotaE[:, :, e], float(e))
            iotaEb = iotaE[:].to_broadcast([P, TO, E])
            m1t = main.tile([P, TO, 1], dt.float32)
            nc.vector.tensor_reduce(out=m1t[:, :, 0], in_=L[:],
                                    op=ALU.max, axis=mybir.AxisListType.X)
            eq1 = main.tile([P, TO, E], dt.float32)
            nc.vector.tensor_tensor(eq1[:], L[:],
                                    m1t[:].to_broadcast([P, TO, E]),
                                    ALU.is_equal)
            sel = main.tile([P, TO, E], dt.float32, tag="sel")
            nc.vector.tensor_tensor(sel[:], eq1[:], iotaEb, ALU.mult)
            nc.vector.tensor_scalar(eq1[:], eq1[:], 1.0, None, ALU.subtract)
            nc.vector.tensor_scalar(eq1[:], eq1[:], -BIGC, None, ALU.mult)
            nc.vector.tensor_add(sel[:], sel[:], eq1[:])
            If = main.tile([P, TO, 2], dt.float32)
            nc.vector.tensor_reduce(out=If[:, :, 0], in_=sel[:],
                                    op=ALU.min, axis=mybir.AxisListType.X)
            # mask out argmax position, find second max
            eqi = main.tile([P, TO, E], dt.float32, tag="eqi")
            nc.vector.tensor_tensor(eqi[:], iotaEb,
                                    If[:, :, 0:1].to_broadcast([P, TO, E]),
                                    ALU.is_equal)
            L2 = main.tile([P, TO, E], dt.float32)
            nc.vector.tensor_scalar(eqi[:], eqi[:], BIGC, None, ALU.mult)
            nc.vector.tensor_sub(L2[:], L[:], eqi[:])
            m2t = main.tile([P, TO, 1], dt.float32)
            nc.vector.tensor_reduce(out=m2t[:, :, 0], in_=L2[:],
                                    op=ALU.max, axis=mybir.AxisListType.X)
            nc.vector.tensor_tensor(eq1[:], L2[:],
                                    m2t[:].to_broadcast([P, TO, E]),
                                    ALU.is_equal)
            nc.vector.tensor_tensor(sel[:], eq1[:], iotaEb, ALU.mult)
            nc.vector.tensor_scalar(eq1[:], eq1[:], 1.0, None, ALU.subtract)
            nc.vector.tensor_scalar(eq1[:], eq1[:], -BIGC, None, ALU.mult)
            nc.vector.tensor_add(sel[:], sel[:], eq1[:])
            nc.vector.tensor_reduce(out=If[:, :, 1], in_=sel[:],
                                    op=ALU.min, axis=mybir.AxisListType.X)
            m1 = m1t[:, :, 0]
            m2 = m2t[:, :, 0]

            expL = main.tile([P, TO, E], dt.float32)
            nc.scalar.activation(expL[:], L[:], AF.Exp)
            Z = main.tile([P, TO], dt.float32)
            nc.vector.reduce_sum(Z[:], expL[:], axis=mybir.AxisListType.X)
            E1 = main.tile([P, TO], dt.float32)
            E2 = main.tile([P, TO], dt.float32)
            nc.scalar.activation(E1[:], m1, AF.Exp)
            nc.scalar.activation(E2[:], m2, AF.Exp)
            rZ = main.tile([P, TO], dt.float32)
            nc.vector.reciprocal(rZ[:], Z[:])
            arg = main.tile([P, TO], dt.float32)
            nc.vector.tensor_sub(arg[:], E1[:], E2[:])
            nc.vector.tensor_mul(arg[:], arg[:], rZ[:])
            w1 = main.tile([P, TO], dt.float32)
            nc.scalar.activation(w1[:], arg[:], AF.Sigmoid)

            mask1 = main.tile([P, TO], dt.float32)
            mask2 = main.tile([P, TO], dt.float32)
            nc.vector.tensor_scalar(mask1[:], If[:, :, 0], eid_sb[:], None,
                                    ALU.is_equal)
            nc.vector.tensor_scalar(mask2[:], If[:, :, 1], eid_sb[:], None,
                                    ALU.is_equal)
            mask = main.tile([P, TO], dt.float32)
            nc.vector.tensor_add(mask[:], mask1[:], mask2[:])
            wsel = main.tile([P, TO], dt.float32)
            tmp = main.tile([P, TO], dt.float32, tag="tmp")
            nc.vector.tensor_mul(wsel[:], mask1[:], w1[:])
            nc.vector.tensor_mul(tmp[:], mask2[:], w1[:])
            nc.vector.tensor_add(wsel[:], wsel[:], mask2[:])
            nc.vector.tensor_sub(wsel[:], wsel[:], tmp[:])

            # --- one-hot expert grid + bucket-rank cumsum ----------------
            # Gf[p, e, o] = [e in top2(t)], t = o*128 + p
            Gf = main.tile([P, E, TO], dt.float32)
            gtmp = main.tile([P, TO], dt.float32, tag="gtmp")
            for e in range(E):
                nc.vector.tensor_scalar(Gf[:, e], If[:, :, 0], float(e), None,
                                        ALU.is_equal)
                nc.vector.tensor_scalar(gtmp[:], If[:, :, 1], float(e), None,
                                        ALU.is_equal)
                nc.vector.tensor_add(Gf[:, e], Gf[:, e], gtmp[:])
            Gb = main.tile([P, E, TO], dt.bfloat16)
            nc.vector.tensor_copy(Gb[:], Gf[:])

            with tc.tile_pool(name="ps_g", bufs=1, space="PSUM") as psg:
                ps_cg = psg.tile([P, E * TO], dt.float32, name="ps_cg")
                nc.tensor.matmul(ps_cg[:], triu_bf[:],
                                 Gb[:].rearrange("p e o -> p (e o)"),
                                 start=True, stop=True)
                csG = main.tile([P, E, TO], dt.float32)
                nc.vector.tensor_copy(csG[:].rearrange("p e o -> p (e o)"),
                                      ps_cg[:])
                ps_ct = psg.tile([1, E * TO], dt.float32, name="ps_ct")
                nc.tensor.matmul(ps_ct[:], ones_bf[:],
                                 Gb[:].rearrange("p e o -> p (e o)"),
                                 start=True, stop=True)
                ctG = main.tile([1, E, 8, SO], dt.float32)
                nc.vector.tensor_copy(
                    ctG[:].rearrange("a e g j -> a (e g j)"), ps_ct[:])

            # column prefix within each home group of SO columns
            offG = main.tile([1, E, 8, SO], dt.float32)
            nc.vector.memset(offG[:, :, :, 0], 0.0)
            nc.vector.tensor_copy(offG[:, :, :, 1], ctG[:, :, :, 0])
            for j in range(2, SO):
                nc.vector.tensor_add(offG[:, :, :, j], offG[:, :, :, j - 1],
                                     ctG[:, :, :, j - 1])
            offGb = main.tile([P, E, TO], dt.float32)
            nc.gpsimd.partition_broadcast(
                offGb[:].rearrange("p e o -> p (e o)"),
                offG[:].rearrange("a e g j -> a (e g j)"))

            # rank[p, e, o]: exclusive rank of token t in bucket (e, home(t))
            rank = main.tile([P, E, TO], dt.float32)
            nc.vector.tensor_add(rank[:], csG[:], offGb[:])
            nc.vector.tensor_sub(rank[:], rank[:], Gf[:])

            # --- expert-side send destination (token grid) ---------------
            # dest = home(t)*CAPR + rank[:, eid, :]
            rank_own = main.tile([P, TO], dt.float32)
            nc.vector.memset(rank_own[:], 0.0)
            etmp = main.tile([P, 1], dt.float32, tag="etmp")
            etmp2 = main.tile([P, TO], dt.float32, tag="etmp2")
            for e in range(E):
                nc.vector.tensor_scalar(etmp[:], eid_sb[:], float(e), None,
                                        ALU.is_equal)
                nc.vector.tensor_scalar(etmp2[:], rank[:, e], etmp[:], None,
                                        ALU.mult)
                nc.vector.tensor_add(rank_own[:], rank_own[:], etmp2[:])
            colc = main.tile([1, TO], dt.float32)
            for g in range(8):
                nc.vector.memset(colc[:, g * SO:(g + 1) * SO], float(g * CAPR))
            colcb = main.tile([P, TO], dt.float32)
            nc.gpsimd.partition_broadcast(colcb[:], colc[:])
            dest_tok = main.tile([P, TO], dt.float32)
            nc.vector.tensor_add(dest_tok[:], rank_own[:], colcb[:])

            # --- home-side gather indices (global table) -----------------
            # idx2[p, o, k] = If_k*CAPR + rank[:, If_k, :]
            idx2 = main.tile([P, TO, 2], dt.float32)
            seltmp = main.tile([P, TO], dt.float32, tag="seltmp")
            for k in range(2):
                nc.vector.tensor_scalar(idx2[:, :, k], If[:, :, k],
                                        float(CAPR), None, ALU.mult)
                for e in range(E):
                    nc.vector.tensor_scalar(seltmp[:], If[:, :, k], float(e),
                                            None, ALU.is_equal)
                    nc.vector.tensor_mul(seltmp[:], seltmp[:], rank[:, e])
                    nc.vector.tensor_add(idx2[:, :, k], idx2[:, :, k],
                                         seltmp[:])
            # quad-packed: gtab4[p + 128h, j*2+k] = idx2[p, 4h+j, k]
            nc.sync.dma_start(
                gtab4[:].rearrange("(h p) (j k) -> p h j k", p=P, j=SO, k=2),
                idx2[:].rearrange("p (h j) k -> p h j k", j=SO))

            # extract own slice rows with ONE eid-computed indirect read
            own32 = main.tile([P, 2, SO], dt.int32)
            eid128 = main.tile([P, 1], dt.float32)
            nc.vector.tensor_scalar(eid128[:], eid_sb[:], float(P), None,
                                    ALU.mult)
            row32 = main.tile([P, 1], dt.int32)
            nc.vector.tensor_copy(row32[:], eid128[:])
            nc.vector.tensor_tensor(row32[:], row32[:], pidx[:], ALU.add)
            ownq = main.tile([P, SO, 2], dt.float32)
            nc.gpsimd.indirect_dma_start(
                out=ownq[:].rearrange("p j k -> p (j k)"), out_offset=None,
                in_=gtab4[:],
                in_offset=bass.IndirectOffsetOnAxis(ap=row32[:, :1], axis=0))
            for k in range(2):
                for o in range(SO):
                    nc.vector.tensor_copy(own32[:, k, o:o + 1],
                                          ownq[:, o, k:k + 1])

            # --- token -> slot compaction (expert side) ------------------
            with tc.tile_pool(name="ps_cs", bufs=1, space="PSUM") as pscs_pool:
                maskb = main.tile([P, TO], dt.bfloat16)
                nc.vector.tensor_copy(maskb[:], mask[:])
                ps_cs = pscs_pool.tile([P, TO], dt.float32)
                nc.tensor.matmul(ps_cs[:], triu_bf[:], maskb[:], start=True,
                                 stop=True)
                csum = main.tile([P, TO], dt.float32)
                nc.vector.tensor_copy(csum[:], ps_cs[:])
                ps_tot = pscs_pool.tile([1, TO], dt.float32, name="ps_tot")
                nc.tensor.matmul(ps_tot[:], ones_bf[:], maskb[:], start=True,
                                 stop=True)
                coltot = main.tile([1, TO], dt.float32)
                nc.vector.tensor_copy(coltot[:], ps_tot[:])
            sc_a = main.tile([1, TO], dt.float32, tag="sca")
            sc_b = main.tile([1, TO], dt.float32, tag="scb")
            nc.vector.tensor_copy(sc_a[:], coltot[:])
            cur, nxt = sc_a, sc_b
            s = 1
            while s < TO:
                nc.vector.tensor_copy(nxt[:], cur[:])
                nc.vector.tensor_add(nxt[:, s:], cur[:, s:], cur[:, :TO - s])
                cur, nxt = nxt, cur
                s *= 2
            offs = main.tile([1, TO], dt.float32)
            nc.vector.memset(offs[:, 0:1], 0.0)
            nc.vector.tensor_copy(offs[:, 1:], cur[:, :TO - 1])
            offs_b = main.tile([P, TO], dt.float32)
            nc.gpsimd.partition_broadcast(offs_b[:], offs[:])

            pos = main.tile([P, TO], dt.float32)
            nc.vector.tensor_add(pos[:], csum[:], offs_b[:])
            nc.vector.tensor_sub(pos[:], pos[:], mask[:])
            trashv = main.tile([P, 1], dt.int32)
            nc.vector.tensor_scalar(trashv[:], pidx[:], 63, None,
                                    ALU.bitwise_and)
            trashf = main.tile([P, 1], dt.float32)
            nc.vector.tensor_copy(trashf[:], trashv[:])
            nc.vector.tensor_scalar(trashf[:], trashf[:], float(C), None, ALU.add)
            slot = main.tile([P, TO], dt.float32)
            nc.vector.tensor_scalar(slot[:], pos[:], trashf[:], None,
                                    ALU.subtract)
            nc.vector.tensor_mul(slot[:], slot[:], mask[:])
            nc.vector.tensor_scalar(slot[:], slot[:], trashf[:], None, ALU.add)
            slot16 = main.tile([P, TO], dt.int16)
            nc.vector.tensor_copy(slot16[:], slot[:])

            # wrapped-by-16 slot list via DRAM roundtrip
            nc.sync.dma_start(slotd[:].rearrange("(o p) -> p o", p=P), slot16[:])
            slot16w = main.tile([P, T // 16], dt.int16)
            slotd_w = slotd[:].rearrange("(cw pw) -> pw cw", pw=16)
            for rep in range(8):
                nc.sync.dma_start(slot16w[rep * 16:(rep + 1) * 16, :], slotd_w)

            # per-token payload rows [tid, wsel, 1, dest, 0...]: 64 f32
            tid32 = main.tile([P, TO], dt.int32)
            nc.gpsimd.iota(tid32[:], pattern=[[P, TO]], base=0,
                           channel_multiplier=1)
            rows = main.tile([P, TO, 16], dt.float32)
            nc.vector.memset(rows[:], 0.0)
            nc.vector.tensor_copy(rows[:, :, 0], tid32[:])
            nc.vector.tensor_copy(rows[:, :, 1], wsel[:])
            nc.vector.memset(rows[:, :, 2], 1.0)
            nc.vector.tensor_copy(rows[:, :, 3], dest_tok[:])

            # scatter the payload rows into slot order
            for k in range(TO // 4):
                nc.gpsimd.dma_scatter_add(
                    out_ap=ilist2[:, :16], in_ap=rows[:, 4 * k:4 * (k + 1), :],
                    idxs_ap=slot16w[:, k * 32:(k + 1) * 32],
                    num_idxs=4 * P, num_idxs_reg=4 * P, elem_size=16,
                    elem_step=64)

            # load back per-slot: tid + weight + dest
            lb = main.tile([P, CM, 4], dt.float32)
            nc.sync.dma_start(
                lb[:], ilist2[:C, :4].rearrange("(m p) c -> p m c", p=P))
            # empty slots (count==0): tid -> T, dest -> trash rows
            em = main.tile([P, CM], dt.float32)
            nc.vector.tensor_scalar(em[:], lb[:, :, 2], 1.0, None,
                                    ALU.subtract)  # count-1: 0 or -1
            tidf = main.tile([P, CM], dt.float32)
            nc.vector.tensor_scalar(tidf[:], em[:], float(-T), None, ALU.mult)
            nc.vector.tensor_add(tidf[:], tidf[:], lb[:, :, 0])
            wlist = main.tile([P, CM], dt.float32)
            nc.vector.tensor_copy(wlist[:], lb[:, :, 1])
            destf = main.tile([P, CM], dt.float32)
            nc.vector.tensor_scalar(destf[:], em[:], float(-SBR), None,
                                    ALU.mult)
            nc.vector.tensor_add(destf[:], destf[:], lb[:, :, 3])
            dest32m = main.tile([P, CM], dt.int32)
            nc.vector.tensor_copy(dest32m[:], destf[:])
            tclf = main.tile([P, CM], dt.float32)
            nc.vector.tensor_scalar(tclf[:], tidf[:], float(T - 1), None,
                                    ALU.min)
            tcl32 = main.tile([P, CM], dt.int32)
            nc.vector.tensor_copy(tcl32[:], tclf[:])

            if stage == "head":
                dummy = main.tile([P, D], dt.float32, tag="dummy")
                nc.vector.tensor_copy(dummy[:, 0:CM], destf[:])
                out_rh = out[:].rearrange("(o p) d -> p o d", p=P)
                for o in range(SO):
                    nc.sync.dma_start(out_rh[:, o], dummy[:])
                continue

            # ------- gather x rows (indirect) + PE-transpose to xeT ------
            mmp = tc.tile_pool(name="mmp", bufs=1)
            mmpool = mmp.__enter__()
            xeT = mmpool.tile([P, KO, C], dt.bfloat16)
            with tc.tile_pool(name="xep", bufs=3) as xep, \
                 tc.tile_pool(name="psT", bufs=4, space="PSUM") as psT:
                for tm in range(CM):
                    xe = xep.tile([P, D], dt.bfloat16, tag="xe", name="xe")
                    nc.gpsimd.indirect_dma_start(
                        out=xe[:], out_offset=None,
                        in_=xh[:],
                        in_offset=bass.IndirectOffsetOnAxis(
                            ap=tcl32[:, tm:tm + 1], axis=0))
                    for ko in range(KO):
                        pt = psT.tile([P, P], dt.bfloat16, tag="pt",
                                      name="pt")
                        nc.tensor.transpose(pt[:], xe[:, ko * P:(ko + 1) * P],
                                            ident_bf[:])
                        nc.scalar.copy(
                            xeT[:, ko, tm * P:(tm + 1) * P], pt[:])

            # ---------------- gate/up matmuls + fuse (chunk-major) -------
            fuse = mmpool.tile([P, FO, C], dt.bfloat16)
            with tc.tile_pool(name="psgu", bufs=4, space="PSUM") as psgu:
                for i in range(3):
                    for fo in range(FO):
                        gb = psgu.tile([P, GC], dt.float32, tag="g", name="g")
                        ub = psgu.tile([P, GC], dt.float32, tag="u", name="u")
                        for ko in range(KO):
                            st = ko == 0
                            sp = ko == KO - 1
                            nc.tensor.matmul(gb[:],
                                             wg_sb[:, ko, fo * P:(fo + 1) * P],
                                             xeT[:, ko, i * GC:(i + 1) * GC],
                                             start=st, stop=sp)
                            nc.tensor.matmul(ub[:],
                                             wu_sb[:, ko, fo * P:(fo + 1) * P],
                                             xeT[:, ko, i * GC:(i + 1) * GC],
                                             start=st, stop=sp)
                        sil = mmpool.tile([P, GC], dt.float32, tag="sil")
                        # silu(g)*u = g*sigmoid(g)*u
                        nc.scalar.activation(sil[:], gb[:], AF.Sigmoid)
                        nc.vector.tensor_mul(sil[:], sil[:], gb[:])
                        nc.vector.tensor_mul(fuse[:, fo, i * GC:(i + 1) * GC],
                                             sil[:], ub[:])

            if stage == "gateup":
                dummy = main.tile([P, D], dt.float32, tag="dummy")
                nc.vector.tensor_copy(dummy[:, 0:GC], fuse[:, 0, 0:GC])
                out_rh = out[:].rearrange("(o p) d -> p o d", p=P)
                for o in range(SO):
                    nc.sync.dma_start(out_rh[:, o], dummy[:])
                mmp.__exit__(None, None, None)
                continue

            # ---------------- down matmul + scale + send-scatter ---------
            dchunks = [(0, 512), (512, 512), (1024, 512), (1536, 512)]
            with tc.tile_pool(name="psd", bufs=2, space="PSUM") as psd, \
                 tc.tile_pool(name="doutp", bufs=3) as doutp:
                for tm in range(CM):
                    dbank = [psd.tile([P, 512], dt.float32, tag=f"d{i}", name=f"d{i}")
                             for i in range(4)]
                    for fo in range(FO):
                        for i, (d0, n) in enumerate(dchunks):
                            nc.tensor.matmul(dbank[i][:],
                                             fuse[:, fo, tm * P:(tm + 1) * P],
                                             wd_sb[:, fo, d0:d0 + n],
                                             start=(fo == 0), stop=(fo == FO - 1))
                    dout = doutp.tile([P, 1, D], dt.bfloat16, tag="dout")
                    for i, (d0, n) in enumerate(dchunks):
                        nc.vector.tensor_scalar(dout[:, 0, d0:d0 + n],
                                                dbank[i][:],
                                                wlist[:, tm:tm + 1], None,
                                                ALU.mult)
                    # plain indirect row-scatter (HW DGE) into sendbuf
                    nc.gpsimd.indirect_dma_start(
                        out=sendbuf[:], out_offset=bass.IndirectOffsetOnAxis(
                            ap=dest32m[:, tm:tm + 1], axis=0),
                        in_=dout[:, 0, :], in_offset=None)

            # ---------------- all-to-all + home combine ------------------
            if n_cores > 1:
                nc.gpsimd.collective_compute(
                    "AllToAll", ALU.bypass,
                    ins=[sendbuf[:SBR].opt()], outs=[recvbuf[:].opt()],
                    replica_groups=[list(range(n_cores))])
            else:
                nc.sync.dma_start(recvbuf[:], sendbuf[:SBR])
            mmp.__exit__(None, None, None)

            out_r = out[:].rearrange("(o p) d -> p o d", p=P)
            with tc.tile_pool(name="finp", bufs=1) as finp:
                gout = finp.tile([P, 2, SO, D], dt.bfloat16)
                g32 = finp.tile([P, 2, SO], dt.int32)
                nc.vector.tensor_copy(g32[:], own32[:])
                for k in range(2):
                    for o in range(SO):
                        nc.gpsimd.indirect_dma_start(
                            out=gout[:, k, o], out_offset=None,
                            in_=recvbuf[:],
                            in_offset=bass.IndirectOffsetOnAxis(
                                ap=g32[:, k, o:o + 1], axis=0))
                for o in range(SO):
                    fin32 = finp.tile([P, D], dt.float32, tag="fin32",
                                      name="fin32")
                    nc.vector.tensor_add(fin32[:], gout[:, 0, o],
                                         gout[:, 1, o])
                    nc.sync.dma_start(out_r[:, o], fin32[:])

    nc.compile()
    return nc


_NC_CACHE = {}


def _get_nc():
    if "nc" not in _NC_CACHE:
        _NC_CACHE["nc"] = build(n_cores=8)
    return _NC_CACHE["nc"]


def make_in_maps(x, router_kernel, w_gate, w_up, w_down):
    import ml_dtypes
    bf16 = ml_dtypes.bfloat16
    x = np.ascontiguousarray(np.asarray(x, dtype=np.float32))
    rk = np.ascontiguousarray(np.asarray(router_kernel, dtype=np.float32))
    wg = np.asarray(w_gate, dtype=np.float32)
    wu = np.asarray(w_up, dtype=np.float32)
    wd = np.asarray(w_down, dtype=np.float32)

    xh = x.astype(bf16)
    xl = (x - xh.astype(np.float32)).astype(bf16)
    rkh = rk.astype(bf16)
    rkl = (rk - rkh.astype(np.float32)).astype(bf16)
    wgh = wg.astype(bf16)
    wuh = wu.astype(bf16)
    wdh = wd.astype(bf16)

    TS = T // 8
    in_maps = []
    for c in range(8):
        in_maps.append({
            "xh": xh,
            "xhs": np.ascontiguousarray(xh[c * TS:(c + 1) * TS]),
            "xls": np.ascontiguousarray(xl[c * TS:(c + 1) * TS]),
            "rkh": rkh,
            "rkl": rkl,
            "wg": np.ascontiguousarray(wgh[c]),
            "wu": np.ascontiguousarray(wuh[c]),
            "wd": np.ascontiguousarray(wdh[c]),
            "eid": np.full((P, 1), float(c), np.float32),
        })
    return in_maps


def kernel(x, router_kernel, w_gate, w_up, w_down):
    """Full-input MoE forward on 8 TRN2 NeuronCores (expert-parallel)."""
    from concourse.bass_utils import run_bass_kernel_spmd

    nc = _get_nc()
    in_maps = make_in_maps(x, router_kernel, w_gate, w_up, w_down)
    res = run_bass_kernel_spmd(nc, in_maps, core_ids=list(range(8)))
    out = np.concatenate([res.results[c]["out"] for c in range(8)], axis=0)
    return out.astype(np.float32)

